# revision 31
# baseline (speedup 1.0000x reference)
"""Trainium2 Bass kernel for nn_NewsEntityGNN (2-layer GraphSAGE + BatchNorm).

Math (per reference):
  h  = relu(BN0(mean_agg(x) @ W_l0 + x @ W_r0))      # biases drop out under BN
  out = BN1(mean_agg(h) @ W_l1 + h @ W_r1)
  BN uses batch statistics over all 50000 nodes (biased var), eps=1e-5.

Distribution: nodes are range-partitioned across 8 NeuronCores (6250 each).
Each core aggregates the edges whose destination it owns:
  - edges grouped on host by 64-node destination windows, split by source
    range (lo: src<32768 / hi: src>=32768 to satisfy int16 gather indices),
    padded to 128-edge tiles; tile counts equalized across cores so one SPMD
    program serves all 8 cores (per-core shortfall is padded with dstloc=-1
    lanes that contribute nothing).
  - per tile: dma_gather fetches 128 source rows (fp16, 256B) from the
    feature table in HBM; a one-hot matrix S[128 edges, 64 dst] built on DVE
    (iota + is_equal) is the MOVING matmul operand against the stationary
    gathered tile, accumulating aggT[feat, dst] in PSUM directly (no PE
    transpose needed).
  - per 128-dst block: evacuate PSUM with a DVE multiply by the host-
    precomputed replicated 1/deg row (mean normalization), two matmuls with
    the (replicated) weight matrices, BatchNorm stats via ACT accumulators.
  - cross-core: AllReduce for BN statistics, AllGather for the layer-0
    output table that layer 1 gathers from.
"""

import numpy as np

import concourse.bass as bass
import concourse.bacc as bacc
import concourse.tile as tile
from concourse import mybir
from concourse.bass_utils import run_bass_kernel_spmd

# problem shapes (hardcoded per contract)
N_NODES = 50000
N_EDGES = 800000
IN_DIM = 100
HID = 128
EPS = 1e-5

NC = 8
NPC = N_NODES // NC          # 6250 nodes per core
P = 128
W = 64                       # dst window width
NW = (NPC + W - 1) // W      # 98 windows per core
NBLK = (NPC + P - 1) // P    # 49 dst blocks per core
SPLIT = 32768                # lo/hi source split for int16 gather indices
D = 128                      # padded feature dim
import os
TPC = int(os.environ.get("K_TPC", "8"))   # tiles per gather chunk
NTOK = TPC * P
SPKT = os.environ.get("K_SP", "1") == "1"  # dma_gather single_packet
UNROLL = os.environ.get("K_UNROLL", "0") == "1"  # python-loop reps (allows collectives)
SCRATCH = int(os.environ.get("K_SCRATCH", "32768"))  # SWDGE desc ring bytes/partition
NQ = int(os.environ.get("K_NQ", "4"))     # SWDGE queues: 4 measured ~2.7x
                             # faster gathers than 1 on HW. CoreSim's
                             # sem-lane/queue lock check only accepts NQ=1
                             # (Tile assigns DMASW lanes in scheduled order),
                             # so sim scripts override K_NQ=1.

f16 = mybir.dt.float16
f32 = mybir.dt.float32
i16 = mybir.dt.int16


# ---------------------------------------------------------------- host prep

def _build_schedule(edge_index):
    """Group edges by (core, window, src-half); equalize tile counts across
    cores. Returns the common schedule plus per-core gather/dstloc arrays."""
    src_old = np.asarray(edge_index[0], dtype=np.int64)
    dst_old = np.asarray(edge_index[1], dtype=np.int64)
    deg_old = np.bincount(dst_old, minlength=N_NODES)
    rank = np.argsort(-deg_old, kind="stable")       # nodes by in-degree desc
    perm = np.empty(N_NODES, np.int64)               # perm[new_id] = old_id
    caps = np.full(NW, W, np.int64)
    caps[NW - 1] = NPC - (NW - 1) * W                # last window is short
    for c in range(NC):
        core_nodes = rank[c::NC]                     # matched degree profiles
        fill = np.zeros(NW, np.int64)
        w = 0
        for node in core_nodes:
            while fill[w] >= caps[w]:
                w = (w + 1) % NW
            perm[c * NPC + w * W + fill[w]] = node
            fill[w] += 1
            w = (w + 1) % NW
    inv = np.empty(N_NODES, np.int64)
    inv[perm] = np.arange(N_NODES)
    src = inv[src_old]                               # relabeled ids
    dst = inv[dst_old]
    core = dst // NPC
    dloc = dst % NPC
    win = dloc // W
    HNPC = NPC // 2
    s_core, s_loc = src // NPC, src % NPC
    half = (s_loc >= HNPC).astype(np.int64)
    relidx = s_core * HNPC + s_loc - half * HNPC  # index into half-table
    key = (core * NW + win) * 2 + half
    order = np.argsort(key, kind="stable")
    s_src = src_old[order]
    s_rel = relidx[order]
    s_dloc = dloc[order]
    counts = np.bincount(key, minlength=NC * NW * 2).reshape(NC, NW, 2)
    starts = np.zeros(NC * NW * 2 + 1, dtype=np.int64)
    np.cumsum(counts.reshape(-1), out=starts[1:])

    tiles = -(-counts // P)                      # ceil
    T = tiles.max(axis=0)                        # [NW, 2] tiles per group
    T[:, 0] = np.maximum(T[:, 0], 1)             # psum coverage guarantee

    # processing tile sequence (common to all cores)
    proc = []                                    # (stream, stream_pos, block, woff)
    spos = [0, 0]
    for b in range(NBLK):
        for w in (2 * b, 2 * b + 1):
            if w >= NW:
                continue
            woff = (w % 2) * W
            for h in (0, 1):
                for _ in range(int(T[w, h])):
                    proc.append((h, spos[h], b, woff))
                    spos[h] += 1
    n_tiles = [spos[0], spos[1]]                 # lo/hi stream tile counts
    n_chunks = [-(-n_tiles[0] // TPC), -(-n_tiles[1] // TPC)]

    # in-degree (same for both layers); mean divides by max(deg, 1)
    invc = (1.0 / np.maximum(deg_old[perm], 1.0)).astype(np.float16)

    # per-core slot arrays: srcs + dstloc per stream slot
    idx_arrs = [[], []]
    dl_arr = []
    src_arr = []
    for c in range(NC):
        slot_src = [np.zeros(n_chunks[h] * NTOK, np.int64) for h in (0, 1)]
        slot_rel = [np.zeros(n_chunks[h] * NTOK, np.int64) for h in (0, 1)]
        slot_dl = [np.full(n_chunks[h] * NTOK, -1.0, np.float32) for h in (0, 1)]
        for w in range(NW):
            for h in (0, 1):
                g = (c * NW + w) * 2 + h
                cnt = counts[c, w, h]
                base = _stream_base(T, w, h)
                e0 = starts[g]
                sl = slice(base * P, base * P + cnt)
                slot_src[h][sl] = s_src[e0 : e0 + cnt]
                slot_rel[h][sl] = s_rel[e0 : e0 + cnt]
                slot_dl[h][sl] = s_dloc[e0 : e0 + cnt] % W
        # wrapped+replicated int16 index layout per chunk
        for h in (0, 1):
            a = slot_rel[h].astype(np.int16).reshape(n_chunks[h], TPC * P // 16, 16)
            wr = a.transpose(0, 2, 1).reshape(n_chunks[h], 16, TPC * P // 16)
            rep = np.tile(wr, (1, 8, 1)).transpose(1, 0, 2).reshape(P, -1)
            idx_arrs[h].append(np.ascontiguousarray(rep))
        # dstloc in processing-tile order [128, n_proc_tiles]
        dl = np.empty((P, len(proc)), np.float32)
        for j, (h, sp, _b, _wo) in enumerate(proc):
            dl[:, j] = slot_dl[h][sp * P : (sp + 1) * P]
        dl_arr.append(dl.astype(np.float16))
        src_arr.append([slot_src[0], slot_src[1]])  # absolute node ids

    return dict(
        proc=proc, T=T, n_tiles=n_tiles, n_chunks=n_chunks,
        idx_lo=[a[0] for a in zip(idx_arrs[0])], idx_hi=[a[0] for a in zip(idx_arrs[1])],
        dstloc=dl_arr, invc=invc, slot_src=src_arr, perm=perm,
    )


def _stream_base(T, w, h):
    """Stream tile index where window w's half-h group begins."""
    base = int(T[:w, h].sum())
    return base


# ------------------------------------------------------------ device program

def _build_program(sched, reps=1, comm=True, ablate=()):
    """ablate: subset of {"gather", "smm", "post", "phasec"} — timing-mode
    switches that skip program pieces to attribute HW time."""
    proc = sched["proc"]
    n_chunks = sched["n_chunks"]
    NT = len(proc)
    NPC_PAD = NBLK * P

    nc = bacc.Bacc("TRN2", target_bir_lowering=False, num_swdge_queues=NQ,
                   dynamic_dma_scratch_size=SCRATCH)

    xg_lo = nc.dram_tensor("xg_lo", [P, n_chunks[0] * NTOK], f16, kind="ExternalInput")
    xg_hi = nc.dram_tensor("xg_hi", [P, n_chunks[1] * NTOK], f16, kind="ExternalInput")
    idx_lo = nc.dram_tensor("idx_lo", [P, n_chunks[0] * NTOK // 16], i16, kind="ExternalInput")
    idx_hi = nc.dram_tensor("idx_hi", [P, n_chunks[1] * NTOK // 16], i16, kind="ExternalInput")
    dstloc = nc.dram_tensor("dstloc", [P, NT], f16, kind="ExternalInput")
    xT_own = nc.dram_tensor("xT_own", [P, NPC], f16, kind="ExternalInput")
    iota64 = nc.dram_tensor("iota64", [P, W], f16, kind="ExternalInput")
    invc_in = nc.dram_tensor("invc_in", [P, NPC_PAD], f16, kind="ExternalInput")
    id16 = nc.dram_tensor("id16", [P, P], f16, kind="ExternalInput")
    id32 = nc.dram_tensor("id32", [P, P], f32, kind="ExternalInput")
    w_all = nc.dram_tensor("w_all", [P, 4 * P], f16, kind="ExternalInput")  # wl0|wr0|wl1|wr1
    gb = nc.dram_tensor("gb", [P, 4], f32, kind="ExternalInput")  # g0|b0|g1|b1

    out_own = nc.dram_tensor("out_own", [NPC, HID], f32, kind="ExternalOutput")

    HNPC = NPC // 2
    h0_own = nc.dram_tensor("h0_own", [NPC, D], f16)
    h0_fullA = nc.dram_tensor("h0_fullA", [NC * HNPC, D], f16, addr_space="Shared")
    h0_fullB = nc.dram_tensor("h0_fullB", [NC * HNPC, D], f16, addr_space="Shared")
    st_in = [nc.dram_tensor(f"st{l}_in", [P, 2], f32) for l in (0, 1)]
    st_out = [nc.dram_tensor(f"st{l}_out", [P, 2], f32, addr_space="Shared") for l in (0, 1)]

    with tile.TileContext(nc) as tc:
        with (
            tc.tile_pool(name="pers", bufs=1) as pers,
            tc.tile_pool(name="glo", bufs=4) as glo_pool,
            tc.tile_pool(name="ghi", bufs=4) as ghi_pool,
            tc.tile_pool(name="sb", bufs=3) as s_pool,
            tc.tile_pool(name="scr", bufs=2) as scr,
            tc.tile_pool(name="psA", bufs=3, space="PSUM") as psA,
            tc.tile_pool(name="psH", bufs=2, space="PSUM") as psH,
            tc.tile_pool(name="psB", bufs=2, space="PSUM") as psB,
        ):
            # ---- persistent loads ----
            ixl = pers.tile([P, n_chunks[0] * NTOK // 16], i16)
            ixh = pers.tile([P, n_chunks[1] * NTOK // 16], i16)
            dl = pers.tile([P, NT], f16)
            xT = pers.tile([P, NPC], f16)
            iota = pers.tile([P, W], f16)
            invc = pers.tile([P, NPC_PAD], f16)
            idT16 = pers.tile([P, P], f16)
            idT32 = pers.tile([P, P], f32)
            wt = pers.tile([P, 4 * P], f16)
            gbt = pers.tile([P, 4], f32)
            eps_t = pers.tile([P, 1], f32)
            nc.vector.memset(eps_t[:], EPS)
            nc.sync.dma_start(out=ixl[:], in_=idx_lo[:])
            nc.sync.dma_start(out=ixh[:], in_=idx_hi[:])
            nc.sync.dma_start(out=dl[:], in_=dstloc[:])
            nc.sync.dma_start(out=xT[:], in_=xT_own[:])
            nc.sync.dma_start(out=iota[:], in_=iota64[:])
            nc.sync.dma_start(out=invc[:], in_=invc_in[:])
            nc.sync.dma_start(out=idT16[:], in_=id16[:])
            nc.sync.dma_start(out=idT32[:], in_=id32[:])
            nc.sync.dma_start(out=wt[:], in_=w_all[:])
            nc.sync.dma_start(out=gbt[:], in_=gb[:])

            hpre = pers.tile([P, NPC_PAD], f32)
            hT0 = pers.tile([P, NPC_PAD], f16)
            ssum = pers.tile([P, NBLK], f32)
            ssq = pers.tile([P, NBLK], f32)

            qn = [0]  # global gather counter (kept for cross-layer stats)
            def rep_body():
              for layer in (0, 1):
                wl = wt[:, layer * 2 * P : layer * 2 * P + P]
                wr = wt[:, (layer * 2 + 1) * P : (layer * 2 + 2) * P]
                gamma = gbt[:, 2 * layer : 2 * layer + 1]
                beta = gbt[:, 2 * layer + 1 : 2 * layer + 2]

                # ---- phase A: aggregate + dense per block ----
                chunk_buf = [{}, {}]
                pools = [glo_pool, ghi_pool]
                tabs = [h0_fullA[:, :], h0_fullB[:, :]]
                ixs = [ixl, ixh]
                xgs = [xg_lo, xg_hi]
                s_bufs = {}

                GC = 4  # layer-0 stream: chunks per DMA (contiguous layout)

                def get_chunk(h, k):
                    if layer == 0:
                        # host pre-gathered, streamed GC chunks per DMA;
                        # contiguous 2KB*GC per partition
                        grp = k // GC
                        if grp not in chunk_buf[h]:
                            nch = min(GC, n_chunks[h] - grp * GC)
                            buf = pools[h].tile([P, GC * TPC, D], f16, tag=f"g{h}", bufs=2)
                            if "gather" not in ablate:
                                xg = xgs[h]
                                base = xg[:]
                                nc.sync.dma_start(
                                    out=buf[:, 0 : nch * TPC, :],
                                    in_=bass.AP(base.tensor,
                                                base.offset + grp * GC * NTOK,
                                                [base.ap[0], [D, nch * TPC], [1, D]]))
                            chunk_buf[h][grp] = buf
                        buf = chunk_buf[h][grp]
                        off = (k % GC) * TPC
                        return buf[:, off : off + TPC, :]
                    if k not in chunk_buf[h]:
                        buf = pools[h].tile([P, TPC, D], f16, tag=f"g{h}1", bufs=4)
                        if "gather" not in ablate:
                            nc.gpsimd.dma_gather(
                                out_ap=buf[:],
                                in_ap=tabs[h],
                                idxs_ap=ixs[h][:, k * NTOK // 16 : (k + 1) * NTOK // 16],
                                num_idxs=NTOK,
                                num_idxs_reg=NTOK,
                                elem_size=D,
                                single_packet=SPKT,
                                queue_num=qn[0] % NQ,
                            )
                            qn[0] += 1
                        chunk_buf[h][k] = buf
                    return chunk_buf[h][k]

                def get_sbatch(jb):
                    if jb not in s_bufs:
                        nb = min(8, NT - jb * 8)
                        sb_t = s_pool.tile([P, 8, W], f16, tag="S")
                        dsl = dl[:, jb * 8 : jb * 8 + nb]
                        dl_b = bass.AP(dl.tensor, dsl.offset, [dsl.ap[0], dsl.ap[1], [0, W]])
                        io_b = bass.AP(iota.tensor, iota[:].offset,
                                       [iota[:].ap[0], [0, nb], iota[:].ap[1]])
                        nc.vector.tensor_tensor(
                            out=sb_t[:, 0:nb, :], in0=io_b, in1=dl_b,
                            op=mybir.AluOpType.is_equal)
                        s_bufs[jb] = sb_t
                    return s_bufs[jb]

                def emit_post(b, aggT):
                    if "post" in ablate:
                        return
                    nb = min(P, NPC - b * P)
                    aggTs = scr.tile([P, P], f16, tag="aggTs")
                    nc.vector.tensor_tensor(
                        out=aggTs[:, 0:nb], in0=aggT[:, 0:nb],
                        in1=invc[:, b * P : b * P + nb],
                        op=mybir.AluOpType.mult)
                    hps = psH.tile([P, P], f32, tag="h")
                    root = xT if layer == 0 else hT0
                    nc.tensor.matmul(out=hps[:, 0:nb], lhsT=wl[:, :],
                                     rhs=aggTs[:, 0:nb], start=True, stop=False)
                    nc.tensor.matmul(out=hps[:, 0:nb], lhsT=wr[:, :],
                                     rhs=root[:, b * P : b * P + nb],
                                     start=False, stop=True)
                    nc.scalar.activation(
                        out=hpre[:, b * P : b * P + nb], in_=hps[:, 0:nb],
                        func=mybir.ActivationFunctionType.Copy,
                        accum_out=ssum[:, b : b + 1])
                    sqs = scr.tile([P, P], f32, tag="sq")
                    nc.scalar.activation(
                        out=sqs[:, 0:nb], in_=hps[:, 0:nb],
                        func=mybir.ActivationFunctionType.Square,
                        accum_out=ssq[:, b : b + 1])

                jidx = 0
                pending = None
                for b in range(NBLK):
                    aggT = psA.tile([P, P], f32, tag="agg")
                    started = set()
                    j0 = jidx
                    while jidx < NT and proc[jidx][2] == b:
                        h, sp, _b, wo = proc[jidx]
                        g = get_chunk(h, sp // TPC)
                        if "smm" in ablate:
                            jidx += 1
                            continue
                        s_t = get_sbatch(jidx // 8)
                        last = (jidx + 1 >= NT or proc[jidx + 1][2] != b
                                or proc[jidx + 1][3] != wo)
                        st = wo not in started
                        started.add(wo)
                        nc.tensor.matmul(
                            out=aggT[:, wo : wo + W],
                            lhsT=g[:, sp % TPC, :],
                            rhs=s_t[:, jidx % 8, :],
                            start=st, stop=last)
                        jidx += 1
                    assert jidx > j0, f"block {b} has no tiles"
                    if pending is not None:
                        emit_post(*pending)
                    pending = (b, aggT)
                emit_post(*pending)

                # ---- phase B: global BN stats ----
                if "post" in ablate:
                    continue
                stats = scr.tile([P, 2], f32, tag="stats")
                nc.vector.tensor_reduce(
                    out=stats[:, 0:1], in_=ssum[:, 0:NBLK],
                    op=mybir.AluOpType.add, axis=mybir.AxisListType.X)
                nc.vector.tensor_reduce(
                    out=stats[:, 1:2], in_=ssq[:, 0:NBLK],
                    op=mybir.AluOpType.add, axis=mybir.AxisListType.X)
                gst = scr.tile([P, 2], f32, tag="gst")
                if comm:
                    nc.sync.dma_start(out=st_in[layer][:], in_=stats[:])
                    nc.gpsimd.collective_compute(
                        "AllReduce", mybir.AluOpType.add,
                        ins=[st_in[layer][:]], outs=[st_out[layer][:]],
                        replica_groups=[list(range(NC))])
                    nc.sync.dma_start(out=gst[:], in_=st_out[layer][:])
                else:
                    # timing mode: skip the collective, use local stats scaled
                    # by NC to keep magnitudes comparable
                    nc.scalar.activation(out=gst[:], in_=stats[:],
                                         func=mybir.ActivationFunctionType.Copy,
                                         scale=float(NC))

                mean = scr.tile([P, 1], f32, tag="mean")
                e2 = scr.tile([P, 1], f32, tag="e2")
                msq = scr.tile([P, 1], f32, tag="msq")
                var = scr.tile([P, 1], f32, tag="var")
                sd = scr.tile([P, 1], f32, tag="sd")
                isd = scr.tile([P, 1], f32, tag="isd")
                a_c = scr.tile([P, 1], f32, tag="a_c")
                mc = scr.tile([P, 1], f32, tag="mc")
                c_c = scr.tile([P, 1], f32, tag="c_c")
                inv_n = 1.0 / float(N_NODES)
                nc.scalar.activation(out=mean[:], in_=gst[:, 0:1],
                                     func=mybir.ActivationFunctionType.Copy, scale=inv_n)
                nc.scalar.activation(out=e2[:], in_=gst[:, 1:2],
                                     func=mybir.ActivationFunctionType.Copy, scale=inv_n)
                nc.scalar.square(out=msq[:], in_=mean[:])
                nc.vector.tensor_sub(out=var[:], in0=e2[:], in1=msq[:])
                nc.vector.tensor_scalar_max(out=var[:], in0=var[:], scalar1=0.0)
                nc.scalar.activation(out=sd[:], in_=var[:],
                                     func=mybir.ActivationFunctionType.Sqrt,
                                     bias=eps_t[:])
                nc.vector.reciprocal(out=isd[:], in_=sd[:])
                nc.vector.tensor_mul(out=a_c[:], in0=gamma[:, :], in1=isd[:])
                nc.vector.tensor_mul(out=mc[:], in0=mean[:], in1=a_c[:])
                nc.vector.tensor_sub(out=c_c[:], in0=beta[:, :], in1=mc[:])

                # ---- phase C: affines first (fills engine queues), then
                # transpose/copy/DMA pipelined across blocks ----
                if "phasec" in ablate:
                    continue
                CHUNK = 2048
                for c0 in range(0, NPC, CHUNK):
                    c1 = min(c0 + CHUNK, NPC)
                    if layer == 0:
                        nc.scalar.activation(
                            out=hT0[:, c0:c1], in_=hpre[:, c0:c1],
                            func=mybir.ActivationFunctionType.Relu,
                            scale=a_c[:], bias=c_c[:])
                    else:
                        cb = bass.AP(c_c.tensor, c_c[:].offset,
                                     [c_c[:].ap[0], [0, c1 - c0]])
                        nc.vector.scalar_tensor_tensor(
                            out=hpre[:, c0:c1], in0=hpre[:, c0:c1],
                            scalar=a_c[:], in1=cb, op0=mybir.AluOpType.mult,
                            op1=mybir.AluOpType.add)
                # transpose groups: G dst blocks share one PSUM bank, then one
                # ACT copy + one (or two, ragged tail) DMA per group
                G = 4 if layer == 0 else 2
                src_t = hT0 if layer == 0 else hpre
                dtyp = f16 if layer == 0 else f32
                identt = idT16 if layer == 0 else idT32
                dst_t = h0_own if layer == 0 else out_own
                dst_w = D if layer == 0 else HID
                for g0 in range(0, NBLK, G):
                    ng = min(G, NBLK - g0)
                    rows = min(ng * P, NPC - g0 * P)
                    nfull = rows // P            # chunks with all 128 rows
                    tail = rows - nfull * P      # rows in ragged last chunk
                    trb = psB.tile([P, G, P], dtyp, tag="tb")
                    for j in range(ng):
                        b = g0 + j
                        nb = min(P, NPC - b * P)
                        nc.tensor.transpose(
                            out=trb[0:nb, j, :],
                            in_=src_t[:, b * P : b * P + nb],
                            identity=identt[:])
                    stg = scr.tile([P, G, P], dtyp, tag="stg")
                    if nfull:
                        nc.scalar.copy(out=stg[:, 0:nfull, :],
                                       in_=trb[:, 0:nfull, :])
                    if tail:
                        nc.scalar.copy(out=stg[0:tail, nfull : nfull + 1, :],
                                       in_=trb[0:tail, nfull : nfull + 1, :])
                    # dram row r = g0*128 + j*128 + p, col f:
                    # AP dims [p: stride dst_w][j: stride 128*dst_w][f: 1]
                    base = dst_t[:]
                    if nfull:
                        nc.sync.dma_start(
                            out=bass.AP(base.tensor, base.offset + g0 * P * dst_w,
                                        [[dst_w, P], [P * dst_w, nfull], [1, dst_w]]),
                            in_=stg[:, 0:nfull, :])
                    if tail:
                        nc.sync.dma_start(
                            out=bass.AP(base.tensor,
                                        base.offset + (g0 + nfull) * P * dst_w,
                                        [[dst_w, tail], [P * dst_w, 1], [1, dst_w]]),
                            in_=stg[0:tail, nfull : nfull + 1, :])
                    if (layer == 0 and comm
                            and g0 * P < HNPC <= (g0 + ng) * P):
                        # first half of h0_own written: gather it to all cores
                        # while the rest of phase C continues; unblocks the
                        # lo-stream layer-1 gathers early
                        nc.gpsimd.collective_compute(
                            "AllGather", mybir.AluOpType.bypass,
                            ins=[h0_own[0:HNPC, :]], outs=[h0_fullA[:]],
                            replica_groups=[list(range(NC))])

                if layer == 0:
                    if comm:
                        nc.gpsimd.collective_compute(
                            "AllGather", mybir.AluOpType.bypass,
                            ins=[h0_own[HNPC:NPC, :]], outs=[h0_fullB[:]],
                            replica_groups=[list(range(NC))])
                    # comm=False: leave the half-tables stale (timing mode)

            if reps == 1:
                rep_body()
            elif UNROLL:
                for _ in range(reps):
                    rep_body()
            else:
                with tc.For_i(0, reps, 1, name="rep"):
                    rep_body()

    nc.compile()
    return nc


# ------------------------------------------------------------------- driver

_CACHE = {}


def _make_in_maps(inputs, sched):
    x = np.asarray(inputs["x"], dtype=np.float32)
    W_l0, W_r0 = inputs["W_l0"], inputs["W_r0"]
    W_l1, W_r1 = inputs["W_l1"], inputs["W_r1"]
    gamma0, beta0 = inputs["gamma0"], inputs["beta0"]
    gamma1, beta1 = inputs["gamma1"], inputs["beta1"]

    x_pad = np.zeros((N_NODES, D), np.float16)
    x_pad[:, :IN_DIM] = x.astype(np.float16)

    def pad_w(w):
        out = np.zeros((P, P), np.float16)
        out[: w.shape[0], : w.shape[1]] = np.asarray(w, dtype=np.float16)
        return out

    w_all = np.concatenate(
        [pad_w(W_l0), pad_w(W_r0), pad_w(W_l1), pad_w(W_r1)], axis=1)
    gb = np.stack([
        np.asarray(gamma0, np.float32), np.asarray(beta0, np.float32),
        np.asarray(gamma1, np.float32), np.asarray(beta1, np.float32)], axis=1)
    iota64 = np.tile(np.arange(W, dtype=np.float16)[None, :], (P, 1))
    ident = np.eye(P, dtype=np.float32)

    NPC_PAD = NBLK * P
    invc = sched["invc"]

    in_maps = []
    for c in range(NC):
        xT = np.zeros((P, NPC), np.float16)
        own = sched["perm"][c * NPC : (c + 1) * NPC]
        xT[:IN_DIM, :] = x[own, :].T.astype(np.float16)
        # layer-0 pre-gather: chunk layout [p][c][f] so each partition's
        # per-chunk bytes are contiguous (2KB HWDGE descriptors)
        xgs = []
        for h in (0, 1):
            srcs = sched["slot_src"][c][h]
            nch = len(srcs) // NTOK
            rows = x_pad[srcs]                       # [nch*NTOK, D]
            rows = rows.reshape(nch, TPC, P, D).transpose(2, 0, 1, 3)
            xgs.append(np.ascontiguousarray(rows.reshape(P, nch * TPC * D)))
        invc_rep = np.zeros((P, NPC_PAD), np.float16)
        invc_rep[:, :NPC] = invc[None, c * NPC : (c + 1) * NPC]
        in_maps.append(dict(
            h0_fullA=np.zeros((N_NODES // 2, D), np.float16),  # pre-zero (sim/
            h0_fullB=np.zeros((N_NODES // 2, D), np.float16),  # timing modes read
            # them before the exchange; ignored by ExternalInput maps)
            xg_lo=xgs[0],
            xg_hi=xgs[1],
            idx_lo=sched["idx_lo"][c],
            idx_hi=sched["idx_hi"][c],
            dstloc=sched["dstloc"][c],
            xT_own=xT,
            iota64=iota64,
            invc_in=invc_rep,
            id16=ident.astype(np.float16),
            id32=ident,
            w_all=w_all,
            gb=gb.astype(np.float32),
        ))
    return in_maps


def kernel(x, edge_index, W_l0, b_l0, W_r0, gamma0, beta0,
           W_l1, b_l1, W_r1, gamma1, beta1):
    edge_index = np.asarray(edge_index)

    sched = _build_schedule(edge_index)
    key = (len(sched["proc"]), sched["n_chunks"][0], sched["n_chunks"][1])
    if key not in _CACHE:
        _CACHE[key] = _build_program(sched)
    nc = _CACHE[key]

    inputs = dict(x=x, W_l0=W_l0, W_r0=W_r0, W_l1=W_l1, W_r1=W_r1,
                  gamma0=gamma0, beta0=beta0, gamma1=gamma1, beta1=beta1)
    in_maps = _make_in_maps(inputs, sched)

    res = run_bass_kernel_spmd(nc, in_maps, list(range(NC)))
    out = np.concatenate([res.results[c]["out_own"] for c in range(NC)], axis=0)
    full = np.empty_like(out)
    full[sched["perm"]] = out                        # new-id rows -> old order
    return full.astype(np.float32)


# revision 32
# speedup vs baseline: 1.3798x; 1.3798x over previous
"""Trainium2 Bass kernel for nn_NewsEntityGNN (2-layer GraphSAGE + BatchNorm).

Math (per reference):
  h  = relu(BN0(mean_agg(x) @ W_l0 + x @ W_r0))      # biases drop out under BN
  out = BN1(mean_agg(h) @ W_l1 + h @ W_r1)
  BN uses batch statistics over all 50000 nodes (biased var), eps=1e-5.

Distribution: nodes are range-partitioned across 8 NeuronCores (6250 each).
Each core aggregates the edges whose destination it owns:
  - edges grouped on host by 64-node destination windows, split by source
    range (lo: src<32768 / hi: src>=32768 to satisfy int16 gather indices),
    padded to 128-edge tiles; tile counts equalized across cores so one SPMD
    program serves all 8 cores (per-core shortfall is padded with dstloc=-1
    lanes that contribute nothing).
  - per tile: dma_gather fetches 128 source rows (fp16, 256B) from the
    feature table in HBM; a one-hot matrix S[128 edges, 64 dst] built on DVE
    (iota + is_equal) is the MOVING matmul operand against the stationary
    gathered tile, accumulating aggT[feat, dst] in PSUM directly (no PE
    transpose needed).
  - per 128-dst block: evacuate PSUM with a DVE multiply by the host-
    precomputed replicated 1/deg row (mean normalization), two matmuls with
    the (replicated) weight matrices, BatchNorm stats via ACT accumulators.
  - cross-core: AllReduce for BN statistics, AllGather for the layer-0
    output table that layer 1 gathers from.
"""

import numpy as np

import concourse.bass as bass
import concourse.bacc as bacc
import concourse.tile as tile
from concourse import mybir
from concourse.bass_utils import run_bass_kernel_spmd

# problem shapes (hardcoded per contract)
N_NODES = 50000
N_EDGES = 800000
IN_DIM = 100
HID = 128
EPS = 1e-5

NC = 8
NPC = N_NODES // NC          # 6250 nodes per core
P = 128
W = 64                       # dst window width
NW = (NPC + W - 1) // W      # 98 windows per core
NBLK = (NPC + P - 1) // P    # 49 dst blocks per core
SPLIT = 32768                # lo/hi source split for int16 gather indices
D = 128                      # padded feature dim
import os
TPC = int(os.environ.get("K_TPC", "8"))   # tiles per gather chunk
NTOK = TPC * P
SPKT = os.environ.get("K_SP", "1") == "1"  # dma_gather single_packet
UNROLL = os.environ.get("K_UNROLL", "0") == "1"  # python-loop reps (allows collectives)
SCRATCH = int(os.environ.get("K_SCRATCH", "32768"))  # SWDGE desc ring bytes/partition
NQ = int(os.environ.get("K_NQ", "4"))     # SWDGE queues: 4 measured ~2.7x
                             # faster gathers than 1 on HW. CoreSim's
                             # sem-lane/queue lock check only accepts NQ=1
                             # (Tile assigns DMASW lanes in scheduled order),
                             # so sim scripts override K_NQ=1.

f16 = mybir.dt.float16
f32 = mybir.dt.float32
i16 = mybir.dt.int16


# ---------------------------------------------------------------- host prep

def _build_schedule(edge_index):
    """Group edges by (core, window, src-half); equalize tile counts across
    cores. Returns the common schedule plus per-core gather/dstloc arrays."""
    src_old = np.asarray(edge_index[0], dtype=np.int64)
    dst_old = np.asarray(edge_index[1], dtype=np.int64)
    deg_old = np.bincount(dst_old, minlength=N_NODES)
    rank = np.argsort(-deg_old, kind="stable")       # nodes by in-degree desc
    perm = np.empty(N_NODES, np.int64)               # perm[new_id] = old_id
    caps = np.full(NW, W, np.int64)
    caps[NW - 1] = NPC - (NW - 1) * W                # last window is short
    for c in range(NC):
        core_nodes = rank[c::NC]                     # matched degree profiles
        fill = np.zeros(NW, np.int64)
        w = 0
        for node in core_nodes:
            while fill[w] >= caps[w]:
                w = (w + 1) % NW
            perm[c * NPC + w * W + fill[w]] = node
            fill[w] += 1
            w = (w + 1) % NW
    inv = np.empty(N_NODES, np.int64)
    inv[perm] = np.arange(N_NODES)
    src = inv[src_old]                               # relabeled ids
    dst = inv[dst_old]
    core = dst // NPC
    dloc = dst % NPC
    win = dloc // W
    half = (src >= SPLIT).astype(np.int64)
    relidx = src - half * SPLIT
    key = (core * NW + win) * 2 + half
    order = np.argsort(key, kind="stable")
    s_src = src_old[order]
    s_rel = relidx[order]
    s_dloc = dloc[order]
    counts = np.bincount(key, minlength=NC * NW * 2).reshape(NC, NW, 2)
    starts = np.zeros(NC * NW * 2 + 1, dtype=np.int64)
    np.cumsum(counts.reshape(-1), out=starts[1:])

    tiles = -(-counts // P)                      # ceil
    T = tiles.max(axis=0)                        # [NW, 2] tiles per group
    T[:, 0] = np.maximum(T[:, 0], 1)             # psum coverage guarantee

    # processing tile sequence (common to all cores)
    proc = []                                    # (stream, stream_pos, block, woff)
    spos = [0, 0]
    for b in range(NBLK):
        for w in (2 * b, 2 * b + 1):
            if w >= NW:
                continue
            woff = (w % 2) * W
            for h in (0, 1):
                for _ in range(int(T[w, h])):
                    proc.append((h, spos[h], b, woff))
                    spos[h] += 1
    n_tiles = [spos[0], spos[1]]                 # lo/hi stream tile counts
    n_chunks = [-(-n_tiles[0] // TPC), -(-n_tiles[1] // TPC)]

    # in-degree (same for both layers); mean divides by max(deg, 1)
    invc = (1.0 / np.maximum(deg_old[perm], 1.0)).astype(np.float16)

    # per-core slot arrays: srcs + dstloc per stream slot
    idx_arrs = [[], []]
    dl_arr = []
    src_arr = []
    for c in range(NC):
        slot_src = [np.zeros(n_chunks[h] * NTOK, np.int64) for h in (0, 1)]
        slot_rel = [np.zeros(n_chunks[h] * NTOK, np.int64) for h in (0, 1)]
        slot_dl = [np.full(n_chunks[h] * NTOK, -1.0, np.float32) for h in (0, 1)]
        for w in range(NW):
            for h in (0, 1):
                g = (c * NW + w) * 2 + h
                cnt = counts[c, w, h]
                base = _stream_base(T, w, h)
                e0 = starts[g]
                sl = slice(base * P, base * P + cnt)
                slot_src[h][sl] = s_src[e0 : e0 + cnt]
                slot_rel[h][sl] = s_rel[e0 : e0 + cnt]
                slot_dl[h][sl] = s_dloc[e0 : e0 + cnt] % W
        # wrapped+replicated int16 index layout per chunk
        for h in (0, 1):
            a = slot_rel[h].astype(np.int16).reshape(n_chunks[h], TPC * P // 16, 16)
            wr = a.transpose(0, 2, 1).reshape(n_chunks[h], 16, TPC * P // 16)
            rep = np.tile(wr, (1, 8, 1)).transpose(1, 0, 2).reshape(P, -1)
            idx_arrs[h].append(np.ascontiguousarray(rep))
        # dstloc in processing-tile order [128, n_proc_tiles]
        dl = np.empty((P, len(proc)), np.float32)
        for j, (h, sp, _b, _wo) in enumerate(proc):
            dl[:, j] = slot_dl[h][sp * P : (sp + 1) * P]
        dl_arr.append(dl.astype(np.float16))
        src_arr.append([slot_src[0], slot_src[1]])  # absolute node ids

    return dict(
        proc=proc, T=T, n_tiles=n_tiles, n_chunks=n_chunks,
        idx_lo=[a[0] for a in zip(idx_arrs[0])], idx_hi=[a[0] for a in zip(idx_arrs[1])],
        dstloc=dl_arr, invc=invc, slot_src=src_arr, perm=perm,
    )


def _stream_base(T, w, h):
    """Stream tile index where window w's half-h group begins."""
    base = int(T[:w, h].sum())
    return base


# ------------------------------------------------------------ device program

def _build_program(sched, reps=1, comm=True, ablate=()):
    """ablate: subset of {"gather", "smm", "post", "phasec"} — timing-mode
    switches that skip program pieces to attribute HW time."""
    proc = sched["proc"]
    n_chunks = sched["n_chunks"]
    NT = len(proc)
    NPC_PAD = NBLK * P

    nc = bacc.Bacc("TRN2", target_bir_lowering=False, num_swdge_queues=NQ,
                   dynamic_dma_scratch_size=SCRATCH)

    xg_lo = nc.dram_tensor("xg_lo", [P, n_chunks[0] * NTOK], f16, kind="ExternalInput")
    xg_hi = nc.dram_tensor("xg_hi", [P, n_chunks[1] * NTOK], f16, kind="ExternalInput")
    idx_lo = nc.dram_tensor("idx_lo", [P, n_chunks[0] * NTOK // 16], i16, kind="ExternalInput")
    idx_hi = nc.dram_tensor("idx_hi", [P, n_chunks[1] * NTOK // 16], i16, kind="ExternalInput")
    dstloc = nc.dram_tensor("dstloc", [P, NT], f16, kind="ExternalInput")
    xT_own = nc.dram_tensor("xT_own", [P, NPC], f16, kind="ExternalInput")
    iota64 = nc.dram_tensor("iota64", [P, W], f16, kind="ExternalInput")
    invc_in = nc.dram_tensor("invc_in", [P, NPC_PAD], f16, kind="ExternalInput")
    id16 = nc.dram_tensor("id16", [P, P], f16, kind="ExternalInput")
    id32 = nc.dram_tensor("id32", [P, P], f32, kind="ExternalInput")
    w_all = nc.dram_tensor("w_all", [P, 4 * P], f16, kind="ExternalInput")  # wl0|wr0|wl1|wr1
    gb = nc.dram_tensor("gb", [P, 4], f32, kind="ExternalInput")  # g0|b0|g1|b1

    out_own = nc.dram_tensor("out_own", [NPC, HID], f32, kind="ExternalOutput")

    h0_own = nc.dram_tensor("h0_own", [NPC, D], f16)
    h0_full = nc.dram_tensor("h0_full", [N_NODES, D], f16, addr_space="Shared")
    st_in = [nc.dram_tensor(f"st{l}_in", [P, 2], f32) for l in (0, 1)]
    st_out = [nc.dram_tensor(f"st{l}_out", [P, 2], f32, addr_space="Shared") for l in (0, 1)]

    with tile.TileContext(nc) as tc:
        with (
            tc.tile_pool(name="pers", bufs=1) as pers,
            tc.tile_pool(name="glo", bufs=4) as glo_pool,
            tc.tile_pool(name="ghi", bufs=4) as ghi_pool,
            tc.tile_pool(name="sb", bufs=3) as s_pool,
            tc.tile_pool(name="scr", bufs=2) as scr,
            tc.tile_pool(name="psA", bufs=3, space="PSUM") as psA,
            tc.tile_pool(name="psH", bufs=2, space="PSUM") as psH,
            tc.tile_pool(name="psB", bufs=2, space="PSUM") as psB,
        ):
            # ---- persistent loads ----
            ixl = pers.tile([P, n_chunks[0] * NTOK // 16], i16)
            ixh = pers.tile([P, n_chunks[1] * NTOK // 16], i16)
            dl = pers.tile([P, NT], f16)
            xT = pers.tile([P, NPC], f16)
            iota = pers.tile([P, W], f16)
            invc = pers.tile([P, NPC_PAD], f16)
            idT16 = pers.tile([P, P], f16)
            idT32 = pers.tile([P, P], f32)
            wt = pers.tile([P, 4 * P], f16)
            gbt = pers.tile([P, 4], f32)
            eps_t = pers.tile([P, 1], f32)
            nc.vector.memset(eps_t[:], EPS)
            nc.sync.dma_start(out=ixl[:], in_=idx_lo[:])
            nc.sync.dma_start(out=ixh[:], in_=idx_hi[:])
            nc.sync.dma_start(out=dl[:], in_=dstloc[:])
            nc.sync.dma_start(out=xT[:], in_=xT_own[:])
            nc.sync.dma_start(out=iota[:], in_=iota64[:])
            nc.sync.dma_start(out=invc[:], in_=invc_in[:])
            nc.sync.dma_start(out=idT16[:], in_=id16[:])
            nc.sync.dma_start(out=idT32[:], in_=id32[:])
            nc.sync.dma_start(out=wt[:], in_=w_all[:])
            nc.sync.dma_start(out=gbt[:], in_=gb[:])

            hpre = pers.tile([P, NPC_PAD], f32)
            hT0 = pers.tile([P, NPC_PAD], f16)
            ssum = pers.tile([P, NBLK], f32)
            ssq = pers.tile([P, NBLK], f32)

            qn = [0]  # global gather counter (kept for cross-layer stats)
            def rep_body():
              for layer in (0, 1):
                wl = wt[:, layer * 2 * P : layer * 2 * P + P]
                wr = wt[:, (layer * 2 + 1) * P : (layer * 2 + 2) * P]
                gamma = gbt[:, 2 * layer : 2 * layer + 1]
                beta = gbt[:, 2 * layer + 1 : 2 * layer + 2]

                # ---- phase A: aggregate + dense per block ----
                chunk_buf = [{}, {}]
                pools = [glo_pool, ghi_pool]
                tabs = [h0_full[0:SPLIT, :], h0_full[SPLIT:N_NODES, :]]
                ixs = [ixl, ixh]
                xgs = [xg_lo, xg_hi]
                s_bufs = {}

                GC = 4  # layer-0 stream: chunks per DMA (contiguous layout)

                def get_chunk(h, k):
                    if layer == 0:
                        # host pre-gathered, streamed GC chunks per DMA;
                        # contiguous 2KB*GC per partition
                        grp = k // GC
                        if grp not in chunk_buf[h]:
                            nch = min(GC, n_chunks[h] - grp * GC)
                            buf = pools[h].tile([P, GC * TPC, D], f16, tag=f"g{h}", bufs=2)
                            if "gather" not in ablate:
                                xg = xgs[h]
                                base = xg[:]
                                nc.sync.dma_start(
                                    out=buf[:, 0 : nch * TPC, :],
                                    in_=bass.AP(base.tensor,
                                                base.offset + grp * GC * NTOK,
                                                [base.ap[0], [D, nch * TPC], [1, D]]))
                            chunk_buf[h][grp] = buf
                        buf = chunk_buf[h][grp]
                        off = (k % GC) * TPC
                        return buf[:, off : off + TPC, :]
                    if k not in chunk_buf[h]:
                        buf = pools[h].tile([P, TPC, D], f16, tag=f"g{h}1", bufs=4)
                        if "gather" not in ablate:
                            nc.gpsimd.dma_gather(
                                out_ap=buf[:],
                                in_ap=tabs[h],
                                idxs_ap=ixs[h][:, k * NTOK // 16 : (k + 1) * NTOK // 16],
                                num_idxs=NTOK,
                                num_idxs_reg=NTOK,
                                elem_size=D,
                                single_packet=SPKT,
                                queue_num=qn[0] % NQ,
                            )
                            qn[0] += 1
                        chunk_buf[h][k] = buf
                    return chunk_buf[h][k]

                def get_sbatch(jb):
                    if jb not in s_bufs:
                        nb = min(8, NT - jb * 8)
                        sb_t = s_pool.tile([P, 8, W], f16, tag="S")
                        dsl = dl[:, jb * 8 : jb * 8 + nb]
                        dl_b = bass.AP(dl.tensor, dsl.offset, [dsl.ap[0], dsl.ap[1], [0, W]])
                        io_b = bass.AP(iota.tensor, iota[:].offset,
                                       [iota[:].ap[0], [0, nb], iota[:].ap[1]])
                        nc.vector.tensor_tensor(
                            out=sb_t[:, 0:nb, :], in0=io_b, in1=dl_b,
                            op=mybir.AluOpType.is_equal)
                        s_bufs[jb] = sb_t
                    return s_bufs[jb]

                def emit_post(b, aggT):
                    if "post" in ablate:
                        return
                    nb = min(P, NPC - b * P)
                    aggTs = scr.tile([P, P], f16, tag="aggTs")
                    nc.vector.tensor_tensor(
                        out=aggTs[:, 0:nb], in0=aggT[:, 0:nb],
                        in1=invc[:, b * P : b * P + nb],
                        op=mybir.AluOpType.mult)
                    hps = psH.tile([P, P], f32, tag="h")
                    root = xT if layer == 0 else hT0
                    nc.tensor.matmul(out=hps[:, 0:nb], lhsT=wl[:, :],
                                     rhs=aggTs[:, 0:nb], start=True, stop=False)
                    nc.tensor.matmul(out=hps[:, 0:nb], lhsT=wr[:, :],
                                     rhs=root[:, b * P : b * P + nb],
                                     start=False, stop=True)
                    nc.scalar.activation(
                        out=hpre[:, b * P : b * P + nb], in_=hps[:, 0:nb],
                        func=mybir.ActivationFunctionType.Copy,
                        accum_out=ssum[:, b : b + 1])
                    sqs = scr.tile([P, P], f32, tag="sq")
                    nc.scalar.activation(
                        out=sqs[:, 0:nb], in_=hps[:, 0:nb],
                        func=mybir.ActivationFunctionType.Square,
                        accum_out=ssq[:, b : b + 1])

                jidx = 0
                pending = None
                for b in range(NBLK):
                    aggT = psA.tile([P, P], f32, tag="agg")
                    started = set()
                    j0 = jidx
                    while jidx < NT and proc[jidx][2] == b:
                        h, sp, _b, wo = proc[jidx]
                        g = get_chunk(h, sp // TPC)
                        if "smm" in ablate:
                            jidx += 1
                            continue
                        s_t = get_sbatch(jidx // 8)
                        last = (jidx + 1 >= NT or proc[jidx + 1][2] != b
                                or proc[jidx + 1][3] != wo)
                        st = wo not in started
                        started.add(wo)
                        nc.tensor.matmul(
                            out=aggT[:, wo : wo + W],
                            lhsT=g[:, sp % TPC, :],
                            rhs=s_t[:, jidx % 8, :],
                            start=st, stop=last)
                        jidx += 1
                    assert jidx > j0, f"block {b} has no tiles"
                    if pending is not None:
                        emit_post(*pending)
                    pending = (b, aggT)
                emit_post(*pending)

                # ---- phase B: global BN stats ----
                if "post" in ablate:
                    continue
                stats = scr.tile([P, 2], f32, tag="stats")
                nc.vector.tensor_reduce(
                    out=stats[:, 0:1], in_=ssum[:, 0:NBLK],
                    op=mybir.AluOpType.add, axis=mybir.AxisListType.X)
                nc.vector.tensor_reduce(
                    out=stats[:, 1:2], in_=ssq[:, 0:NBLK],
                    op=mybir.AluOpType.add, axis=mybir.AxisListType.X)
                gst = scr.tile([P, 2], f32, tag="gst")
                if comm:
                    nc.sync.dma_start(out=st_in[layer][:], in_=stats[:])
                    nc.gpsimd.collective_compute(
                        "AllReduce", mybir.AluOpType.add,
                        ins=[st_in[layer][:]], outs=[st_out[layer][:]],
                        replica_groups=[list(range(NC))])
                    nc.sync.dma_start(out=gst[:], in_=st_out[layer][:])
                else:
                    # timing mode: skip the collective, use local stats scaled
                    # by NC to keep magnitudes comparable
                    nc.scalar.activation(out=gst[:], in_=stats[:],
                                         func=mybir.ActivationFunctionType.Copy,
                                         scale=float(NC))

                mean = scr.tile([P, 1], f32, tag="mean")
                e2 = scr.tile([P, 1], f32, tag="e2")
                msq = scr.tile([P, 1], f32, tag="msq")
                var = scr.tile([P, 1], f32, tag="var")
                sd = scr.tile([P, 1], f32, tag="sd")
                isd = scr.tile([P, 1], f32, tag="isd")
                a_c = scr.tile([P, 1], f32, tag="a_c")
                mc = scr.tile([P, 1], f32, tag="mc")
                c_c = scr.tile([P, 1], f32, tag="c_c")
                inv_n = 1.0 / float(N_NODES)
                nc.scalar.activation(out=mean[:], in_=gst[:, 0:1],
                                     func=mybir.ActivationFunctionType.Copy, scale=inv_n)
                nc.scalar.activation(out=e2[:], in_=gst[:, 1:2],
                                     func=mybir.ActivationFunctionType.Copy, scale=inv_n)
                nc.scalar.square(out=msq[:], in_=mean[:])
                nc.vector.tensor_sub(out=var[:], in0=e2[:], in1=msq[:])
                nc.vector.tensor_scalar_max(out=var[:], in0=var[:], scalar1=0.0)
                nc.scalar.activation(out=sd[:], in_=var[:],
                                     func=mybir.ActivationFunctionType.Sqrt,
                                     bias=eps_t[:])
                nc.vector.reciprocal(out=isd[:], in_=sd[:])
                nc.vector.tensor_mul(out=a_c[:], in0=gamma[:, :], in1=isd[:])
                nc.vector.tensor_mul(out=mc[:], in0=mean[:], in1=a_c[:])
                nc.vector.tensor_sub(out=c_c[:], in0=beta[:, :], in1=mc[:])

                # ---- phase C: affines first (fills engine queues), then
                # transpose/copy/DMA pipelined across blocks ----
                if "phasec" in ablate:
                    continue
                CHUNK = 2048
                for c0 in range(0, NPC, CHUNK):
                    c1 = min(c0 + CHUNK, NPC)
                    if layer == 0:
                        nc.scalar.activation(
                            out=hT0[:, c0:c1], in_=hpre[:, c0:c1],
                            func=mybir.ActivationFunctionType.Relu,
                            scale=a_c[:], bias=c_c[:])
                    else:
                        cb = bass.AP(c_c.tensor, c_c[:].offset,
                                     [c_c[:].ap[0], [0, c1 - c0]])
                        nc.vector.scalar_tensor_tensor(
                            out=hpre[:, c0:c1], in0=hpre[:, c0:c1],
                            scalar=a_c[:], in1=cb, op0=mybir.AluOpType.mult,
                            op1=mybir.AluOpType.add)
                # transpose groups: G dst blocks share one PSUM bank, then one
                # ACT copy + one (or two, ragged tail) DMA per group
                G = 4 if layer == 0 else 2
                src_t = hT0 if layer == 0 else hpre
                dtyp = f16 if layer == 0 else f32
                identt = idT16 if layer == 0 else idT32
                dst_t = h0_own if layer == 0 else out_own
                dst_w = D if layer == 0 else HID
                for g0 in range(0, NBLK, G):
                    ng = min(G, NBLK - g0)
                    rows = min(ng * P, NPC - g0 * P)
                    nfull = rows // P            # chunks with all 128 rows
                    tail = rows - nfull * P      # rows in ragged last chunk
                    trb = psB.tile([P, G, P], dtyp, tag="tb")
                    for j in range(ng):
                        b = g0 + j
                        nb = min(P, NPC - b * P)
                        nc.tensor.transpose(
                            out=trb[0:nb, j, :],
                            in_=src_t[:, b * P : b * P + nb],
                            identity=identt[:])
                    stg = scr.tile([P, G, P], dtyp, tag="stg")
                    if nfull:
                        nc.scalar.copy(out=stg[:, 0:nfull, :],
                                       in_=trb[:, 0:nfull, :])
                    if tail:
                        nc.scalar.copy(out=stg[0:tail, nfull : nfull + 1, :],
                                       in_=trb[0:tail, nfull : nfull + 1, :])
                    # dram row r = g0*128 + j*128 + p, col f:
                    # AP dims [p: stride dst_w][j: stride 128*dst_w][f: 1]
                    base = dst_t[:]
                    if nfull:
                        nc.sync.dma_start(
                            out=bass.AP(base.tensor, base.offset + g0 * P * dst_w,
                                        [[dst_w, P], [P * dst_w, nfull], [1, dst_w]]),
                            in_=stg[:, 0:nfull, :])
                    if tail:
                        nc.sync.dma_start(
                            out=bass.AP(base.tensor,
                                        base.offset + (g0 + nfull) * P * dst_w,
                                        [[dst_w, tail], [P * dst_w, 1], [1, dst_w]]),
                            in_=stg[0:tail, nfull : nfull + 1, :])

                if layer == 0:
                    if comm:
                        nc.gpsimd.collective_compute(
                            "AllGather", mybir.AluOpType.bypass,
                            ins=[h0_own[:]], outs=[h0_full[:]],
                            replica_groups=[list(range(NC))])
                    # comm=False: leave h0_full stale (timing mode)

            if reps == 1:
                rep_body()
            elif UNROLL:
                for _ in range(reps):
                    rep_body()
            else:
                with tc.For_i(0, reps, 1, name="rep"):
                    rep_body()

    nc.compile()
    return nc


# ------------------------------------------------------------------- driver

_CACHE = {}


def _make_in_maps(inputs, sched):
    x = np.asarray(inputs["x"], dtype=np.float32)
    W_l0, W_r0 = inputs["W_l0"], inputs["W_r0"]
    W_l1, W_r1 = inputs["W_l1"], inputs["W_r1"]
    gamma0, beta0 = inputs["gamma0"], inputs["beta0"]
    gamma1, beta1 = inputs["gamma1"], inputs["beta1"]

    x_pad = np.zeros((N_NODES, D), np.float16)
    x_pad[:, :IN_DIM] = x.astype(np.float16)

    def pad_w(w):
        out = np.zeros((P, P), np.float16)
        out[: w.shape[0], : w.shape[1]] = np.asarray(w, dtype=np.float16)
        return out

    w_all = np.concatenate(
        [pad_w(W_l0), pad_w(W_r0), pad_w(W_l1), pad_w(W_r1)], axis=1)
    gb = np.stack([
        np.asarray(gamma0, np.float32), np.asarray(beta0, np.float32),
        np.asarray(gamma1, np.float32), np.asarray(beta1, np.float32)], axis=1)
    iota64 = np.tile(np.arange(W, dtype=np.float16)[None, :], (P, 1))
    ident = np.eye(P, dtype=np.float32)

    NPC_PAD = NBLK * P
    invc = sched["invc"]

    in_maps = []
    for c in range(NC):
        xT = np.zeros((P, NPC), np.float16)
        own = sched["perm"][c * NPC : (c + 1) * NPC]
        xT[:IN_DIM, :] = x[own, :].T.astype(np.float16)
        # layer-0 pre-gather: chunk layout [p][c][f] so each partition's
        # per-chunk bytes are contiguous (2KB HWDGE descriptors)
        xgs = []
        for h in (0, 1):
            srcs = sched["slot_src"][c][h]
            nch = len(srcs) // NTOK
            rows = x_pad[srcs]                       # [nch*NTOK, D]
            rows = rows.reshape(nch, TPC, P, D).transpose(2, 0, 1, 3)
            xgs.append(np.ascontiguousarray(rows.reshape(P, nch * TPC * D)))
        invc_rep = np.zeros((P, NPC_PAD), np.float16)
        invc_rep[:, :NPC] = invc[None, c * NPC : (c + 1) * NPC]
        in_maps.append(dict(
            h0_full=np.zeros((N_NODES, D), np.float16),  # pre-zero (sim/timing
            # modes read it before the exchange; ignored by ExternalInput maps)
            xg_lo=xgs[0],
            xg_hi=xgs[1],
            idx_lo=sched["idx_lo"][c],
            idx_hi=sched["idx_hi"][c],
            dstloc=sched["dstloc"][c],
            xT_own=xT,
            iota64=iota64,
            invc_in=invc_rep,
            id16=ident.astype(np.float16),
            id32=ident,
            w_all=w_all,
            gb=gb.astype(np.float32),
        ))
    return in_maps


def kernel(x, edge_index, W_l0, b_l0, W_r0, gamma0, beta0,
           W_l1, b_l1, W_r1, gamma1, beta1):
    edge_index = np.asarray(edge_index)

    sched = _build_schedule(edge_index)
    key = (len(sched["proc"]), sched["n_chunks"][0], sched["n_chunks"][1])
    if key not in _CACHE:
        _CACHE[key] = _build_program(sched)
    nc = _CACHE[key]

    inputs = dict(x=x, W_l0=W_l0, W_r0=W_r0, W_l1=W_l1, W_r1=W_r1,
                  gamma0=gamma0, beta0=beta0, gamma1=gamma1, beta1=beta1)
    in_maps = _make_in_maps(inputs, sched)

    res = run_bass_kernel_spmd(nc, in_maps, list(range(NC)))
    out = np.concatenate([res.results[c]["out_own"] for c in range(NC)], axis=0)
    full = np.empty_like(out)
    full[sched["perm"]] = out                        # new-id rows -> old order
    return full.astype(np.float32)
